# revision 58
# baseline (speedup 1.0000x reference)
"""1-D peak-IoU NMS (nn_Detector) on 8 Trainium2 NeuronCores.

Algorithm:
  * Only boxes with conf > 0.5 can be kept or suppress others; after the
    confidence sort they form a prefix of M boxes.
  * piou(i,j) is symmetric and piou > 0.5 requires interval overlap, which
    requires |start_i - start_j| < max_width (95).  Sorting the valid boxes
    by start, every relevant pair lies within a forward rank band.  The
    device computes the suppression margin for the first K=96 forward
    neighbors (the realized max suppression offset for this generator
    regime is 83); the host covers the residual band up to the exact
    per-input overlap bound with the same fp32 formula (normally empty),
    so the kernel is correct for any input.
  * Margin: S > 0  <=>  piou > 0.5 via the exact-sign division-free form
    S = (ia - ua/2)*ud - pd*ua   (ua, ud > 0).
  * The inputs are pre-skewed on the host (skw[p, x] = field[base+p+1+x])
    so each [128, K] grid column is a needed pair and the DMA is plain
    contiguous.  Work is split across engines: the Scalar (ACT) engine
    produces min(h_i,h_j) and |p_i - p_j|, DVE the rest; row-sharded over
    8 cores; the host performs the cheap greedy sequential resolution.
"""

import os
import numpy as np

N = 16384
THRESH = 0.5
NCORES = 8
NT = 8                 # 128-row tiles per core
RC = NT * 128          # rows per core
RTOT = NCORES * RC     # padded valid-box capacity (8192)
K = 96                 # device forward band width (realized max offset is 83)
FC = RC + 192          # padded source column span per core
NFIELD = 6             # skw fields: s, e, p, h, a, w
NROWF = 7              # rows fields: s, e, p, h, a, w, -p
NPAIR = NT // 2
K2 = 2 * K
TW = NFIELD * K        # per-tile chunk width in skw layout
ROFF = NT * NROWF      # rows table width, prepended to chunk 0

_cache = {}
last_results = None    # BassKernelResults of the most recent device run


def _build_bass():
    import concourse.bass as bass
    import concourse.mybir as mybir
    from contextlib import ExitStack

    f32 = mybir.dt.float32
    bf16 = mybir.dt.bfloat16
    Alu = mybir.AluOpType
    Act = mybir.ActivationFunctionType
    nc = bass.Bass()
    skw_t = nc.declare_dram_parameter("skw", [128, ROFF + NT * TW], f32, isOutput=False)
    marg_t = nc.declare_dram_parameter("marg", [128, NT * K], bf16, isOutput=True)

    with ExitStack() as ctx:
        skw_sb = ctx.enter_context(
            nc.sbuf_tensor("skw_sb", [128, ROFF + NT * TW], f32)
        )
        out_sb = ctx.enter_context(nc.sbuf_tensor("out_sb", [128, NT * K], bf16))
        pd_sb = ctx.enter_context(nc.sbuf_tensor("pd_sb", [128, NT * K], f32))
        rbuf = ctx.enter_context(nc.sbuf_tensor("rbuf", [128, K], f32))
        # mh is ACT-produced, double-buffered by pair parity
        mh_sb = ctx.enter_context(nc.sbuf_tensor("mh_sb", [128, 2 * K2], f32))
        slab = {
            nm: ctx.enter_context(nc.sbuf_tensor(f"sl_{nm}", [128, K2], f32))
            for nm in ("mxs", "il0", "ia", "ua", "ud", "g", "t1", "t2")
        }
        cin = [ctx.enter_context(nc.semaphore(f"cin{q}")) for q in range(NT)]
        mh_sem = ctx.enter_context(nc.semaphore("mh_sem"))
        pd_sem = ctx.enter_context(nc.semaphore("pd_sem"))
        dve_done = ctx.enter_context(nc.semaphore("dve_done"))
        dma_out = ctx.enter_context(nc.semaphore("dma_out"))
        block = ctx.enter_context(nc.Block())

        def cj(f, t):
            base = ROFF + t * TW + f * K
            return skw_sb[:, base : base + K]

        def ri(f, t):
            return skw_sb[:, t * NROWF + f : t * NROWF + f + 1]

        @block.sync
        def _(sync):
            # chunk t: the contiguous per-tile block of all 6 skewed fields;
            # chunk 0 additionally carries the whole per-row scalar table.
            for q in range(NT):
                lo = 0 if q == 0 else ROFF + q * TW
                hi = ROFF + (q + 1) * TW
                sync.dma_start(
                    out=skw_sb[:, lo:hi], in_=skw_t[:, lo:hi]
                ).then_inc(cin[q], 16)
            for j in range(NPAIR):
                sync.dma_start(
                    out=marg_t[:, j * K2 : (j + 1) * K2],
                    in_=out_sb[:, j * K2 : (j + 1) * K2],
                )._wait_ge(dve_done, j + 1).then_inc(dma_out, 16)
            sync.wait_ge(dma_out, 16 * NPAIR)

        @block.scalar
        def _(scalar):
            # warm the activation table before inputs land
            zero_ap = nc.const_aps.aps[(f32, 0.0)]
            scalar.activation(rbuf[:, :1], zero_ap, Act.Abs, bias=0.0, scale=1.0)

            def emit_mh(j):
                # mh = h_i - relu(h_i - h_j)   == min(h_i, h_j) up to 1 ulp
                par = (j % 2) * K2
                if j >= 2:
                    # parity slab last read by ia of pair j-2
                    scalar.wait_ge(dve_done, j - 1)
                for k, t in enumerate((2 * j, 2 * j + 1)):
                    ins = scalar.activation(
                        rbuf[:], cj(3, t), Act.Relu, bias=ri(3, t), scale=-1.0
                    )
                    ins._wait_ge(cin[t], 16)
                    ins = scalar.activation(
                        mh_sb[:, par + k * K : par + (k + 1) * K],
                        rbuf[:],
                        Act.Abs,
                        bias=ri(3, t),
                        scale=-1.0,
                    )
                ins.then_inc(mh_sem, 1)

            def emit_pd(j, gate=False):
                for k, t in enumerate((2 * j, 2 * j + 1)):
                    ins = scalar.activation(
                        pd_sb[:, t * K : (t + 1) * K],
                        cj(2, t),
                        Act.Abs,
                        bias=ri(6, t),
                        scale=1.0,
                    )
                    if gate:
                        # first ACT data ops: must gate on the input chunks
                        ins._wait_ge(cin[t], 16)
                ins.then_inc(pd_sem, 1)

            # Pair 0's mh is computed on DVE (avoids the startup stall while
            # ACT's serial chain catches up); ACT starts directly with pd(0).
            # mh(j) immediately precedes pd(j) afterwards.
            emit_pd(0, gate=True)
            for j in range(1, NPAIR):
                emit_mh(j)
                emit_pd(j)

        @block.vector
        def _(vector):
            for j in range(NPAIR):
                tiles = range(2 * j, 2 * j + 2)
                par = (j % 2) * K2
                for k, t in enumerate(tiles):
                    h = slice(k * K, (k + 1) * K)
                    # mxs = max(s_j, s_i)
                    ins = vector.tensor_scalar_max(slab["mxs"][:, h], cj(0, t), ri(0, t))
                    ins._wait_ge(cin[t], 16)
                    # il0 = min(e_j, e_i) - mxs
                    vector.scalar_tensor_tensor(
                        slab["il0"][:, h], cj(1, t), ri(1, t), slab["mxs"][:, h],
                        Alu.min, Alu.subtract,
                    )
                if j == 0:
                    # pair 0: mh on DVE itself — no cross-engine wait at startup
                    for k, t in enumerate(tiles):
                        vector.tensor_scalar_min(
                            mh_sb[:, par + k * K : par + (k + 1) * K], cj(3, t), ri(3, t)
                        )
                # ia = relu(il0) * mh            [pair-wide, mh from ACT for j>0]
                ins = vector.scalar_tensor_tensor(
                    slab["ia"][:], slab["il0"][:], 0.0, mh_sb[:, par : par + K2],
                    Alu.max, Alu.mult,
                )
                if j > 0:
                    ins._wait_ge(mh_sem, j)
                for k, t in enumerate(tiles):
                    h = slice(k * K, (k + 1) * K)
                    # ua = (a_j + a_i) - ia      (union area, > 0)
                    vector.scalar_tensor_tensor(
                        slab["ua"][:, h], cj(4, t), ri(4, t), slab["ia"][:, h],
                        Alu.add, Alu.subtract,
                    )
                    # ud = (w_j + w_i) - il0     (union length, > 0)
                    vector.scalar_tensor_tensor(
                        slab["ud"][:, h], cj(5, t), ri(5, t), slab["il0"][:, h],
                        Alu.add, Alu.subtract,
                    )
                # g = ia - 0.5*ua                [quad-wide]
                vector.scalar_tensor_tensor(
                    slab["g"][:], slab["ua"][:], -0.5, slab["ia"][:], Alu.mult, Alu.add
                )
                vector.tensor_mul(slab["t1"][:], slab["g"][:], slab["ud"][:])
                vector.tensor_mul(
                    slab["t2"][:], pd_sb[:, j * K2 : (j + 1) * K2], slab["ua"][:]
                )._wait_ge(pd_sem, j + 1)
                vector.tensor_sub(
                    out_sb[:, j * K2 : (j + 1) * K2], slab["t1"][:], slab["t2"][:]
                ).then_inc(dve_done, 1)
    return nc


def _get_bass():
    if "nc" not in _cache:
        _cache["nc"] = _build_bass()
    return _cache["nc"]


def _prep_core_inputs(fpad):
    """fpad: [NROWF, >=RC*(NCORES-1)+1+FC] padded start-sorted field table."""
    from numpy.lib.stride_tricks import as_strided

    in_maps = []
    for r in range(NCORES):
        base = r * RC
        buf = np.empty((128, ROFF + NT * TW), np.float32)
        # per-row scalar table, prepended to chunk 0
        buf[:, :ROFF] = (
            fpad[:, base : base + RC]
            .reshape(NROWF, NT, 128)
            .transpose(2, 1, 0)
            .reshape(128, NT * NROWF)
        )
        # skw[p, t, f, c] = fpad[f, base + 1 + t*128 + p + c]   (chunk-major)
        c0 = np.ascontiguousarray(
            fpad[:NFIELD, base + 1 : base + 1 + (NT - 1) * 128 + 127 + K]
        )
        sf, sx = c0.strides
        skw = as_strided(
            c0, shape=(128, NT, NFIELD, K), strides=(sx, 128 * sx, sf, sx)
        )
        buf[:, ROFF:] = skw.reshape(128, NT * TW)
        in_maps.append({"skw": buf})
    return in_maps


def _band_from_margins(margs):
    """margs: list of [128, NT*K] per core (bf16) -> B [RTOT, K] band margins.
    bf16 rounding preserves the margin sign (|S| >> bf16 underflow)."""
    B = np.empty((RTOT, K), np.float32)
    for r in range(NCORES):
        m = np.asarray(margs[r]).astype(np.float32).reshape(128, NT, K)
        B[r * RC : (r + 1) * RC] = m.transpose(1, 0, 2).reshape(RC, K)
    return B


def _host_margin(fi, fj):
    """Exact device-op-order fp32 margin for box rows fi vs fj (dicts of arrays)."""
    f32 = np.float32
    mxs = np.maximum(fi["s"], fj["s"])
    il0 = (np.minimum(fi["e"], fj["e"]) - mxs).astype(f32)
    mh = np.minimum(fi["h"], fj["h"])
    ia = (np.maximum(il0, 0) * mh).astype(f32)
    ua = ((fj["a"] + fi["a"]).astype(f32) - ia).astype(f32)
    pd = np.abs((fj["p"] - fi["p"]).astype(f32))
    ud = ((fj["w"] + fi["w"]).astype(f32) - il0).astype(f32)
    g = ((ua * f32(-0.5)).astype(f32) + ia).astype(f32)
    t1 = (g * ud).astype(f32)
    t2 = (pd * ua).astype(f32)
    return (t1 - t2).astype(f32)


def _residual_pairs(flds, M, kr):
    """Suppression pairs with offset in (K, kr] computed on host (normally none)."""
    if M <= K + 1 or kr <= K:
        return np.empty(0, np.int64), np.empty(0, np.int64)
    u = np.arange(M)[:, None]
    d = np.arange(K + 1, kr + 1)[None, :]
    v = u + d
    ok = v < M
    vc = np.clip(v, 0, M - 1)
    fi = {k: flds[k][u] for k in flds}
    fj = {k: flds[k][vc] for k in flds}
    S = _host_margin(fi, fj)
    su, sd = np.nonzero((S > 0) & ok)
    return su, su + sd + K + 1


def _resolve(M, so, uu, vv):
    """Greedy NMS resolution from suppression pairs (start-order indices)."""
    cu, cv = so[uu], so[vv]
    lo = np.minimum(cu, cv)
    hi = np.maximum(cu, cv)
    o = np.argsort(lo, kind="stable")
    lo, hi = lo[o], hi[o]
    starts = np.searchsorted(lo, np.arange(M + 1))
    keep = np.zeros(M, bool)
    removed = np.zeros(M, bool)
    for rk in range(M):
        if not removed[rk]:
            keep[rk] = True
            removed[hi[starts[rk] : starts[rk + 1]]] = True
    return keep


def _clear_backends():
    try:
        import jax.extend.backend as _jeb

        _jeb.clear_backends()
    except Exception:
        try:
            import jax

            jax.clear_backends()
        except Exception:
            pass


def _ensure_devices():
    """If the caller pinned jax to another platform (leaving fewer than
    NCORES visible devices, which would crash the device run anyway), flip
    to the axon platform. Returns the previous jax_platforms value to
    restore, or None if nothing was changed."""
    try:
        import jax

        if len(jax.devices()) >= NCORES:
            return None
        prev = jax.config.jax_platforms
        jax.config.update("jax_platforms", "axon")
        _clear_backends()
        if len(jax.devices()) >= NCORES:
            return prev
        jax.config.update("jax_platforms", prev)
        _clear_backends()
    except Exception:
        pass
    return None


def kernel(output):
    global last_results
    from concourse.bass_utils import run_bass_kernel_spmd

    output = np.asarray(output, dtype=np.float32)
    conf = output[:, 0]
    order = np.argsort(-conf, kind="stable")
    boxes = output[order]
    M = int((boxes[:, 0] > THRESH).sum())
    assert M <= RTOT, f"valid-box count {M} exceeds kernel capacity {RTOT}"

    V = boxes[:M]
    s = V[:, 1].copy()
    e = V[:, 2].copy()
    p = V[:, 3].copy()
    h = V[:, 4].copy()
    w = (e - s).astype(np.float32)
    a = (w * h).astype(np.float32)
    so = np.argsort(s, kind="stable")            # start-order -> conf rank

    # exact per-input overlap bound: boxes more than maxgap ranks apart are
    # disjoint; the host covers offsets (K, maxgap] (normally none fire)
    ss = s[so]
    maxgap = int((np.searchsorted(ss, ss + np.float32(95.0)) - np.arange(M)).max())

    PAD = RC * (NCORES - 1) + 1 + FC
    fpad = np.zeros((NROWF, max(PAD, RTOT)), np.float32)
    fields = np.stack([s[so], e[so], p[so], h[so], a[so], w[so], -p[so]])
    fpad[:, :M] = fields

    nc = _get_bass()
    in_maps = _prep_core_inputs(fpad)
    trace = bool(int(os.environ.get("NMS_TRACE", "0")))
    prev_platforms = _ensure_devices()
    try:
        res = run_bass_kernel_spmd(nc, in_maps, list(range(NCORES)), trace=trace)
        last_results = res
        margs = [np.asarray(res.results[r]["marg"]) for r in range(NCORES)]
    finally:
        if prev_platforms is not None:
            try:
                import jax

                jax.config.update("jax_platforms", prev_platforms)
                _clear_backends()
            except Exception:
                pass

    B = _band_from_margins(margs)
    uu, dd = np.nonzero(B > 0)
    vv = uu + dd + 1
    ok = (uu < M) & (vv < M)
    uu, vv = uu[ok], vv[ok]
    # residual band (K, maxgap] on host — normally empty for this regime
    flds = {k: fields[i][:M] for i, k in enumerate(("s", "e", "p", "h", "a", "w"))}
    ru, rv = _residual_pairs(flds, M, maxgap)
    uu = np.concatenate([uu, ru])
    vv = np.concatenate([vv, rv])

    keepM = _resolve(M, so, uu, vv)
    keep_full = np.zeros(N, bool)
    keep_full[:M] = keepM
    return boxes[:, 1:] * keep_full[:, None].astype(np.float32)


# revision 59
# speedup vs baseline: 1.0282x; 1.0282x over previous
"""1-D peak-IoU NMS (nn_Detector) on 8 Trainium2 NeuronCores.

Algorithm:
  * Only boxes with conf > 0.5 can be kept or suppress others; after the
    confidence sort they form a prefix of M boxes.
  * piou(i,j) is symmetric and piou > 0.5 requires interval overlap, which
    requires |start_i - start_j| < max_width (95).  Sorting the valid boxes
    by start, every relevant pair lies within a forward rank band.  The
    device computes the suppression margin for the first K=96 forward
    neighbors (the realized max suppression offset for this generator
    regime is 83); the host covers the residual band up to the exact
    per-input overlap bound with the same fp32 formula (normally empty),
    so the kernel is correct for any input.
  * Margin: S > 0  <=>  piou > 0.5 via the exact-sign division-free form
    S = (ia - ua/2)*ud - pd*ua   (ua, ud > 0).
  * The inputs are pre-skewed on the host (skw[p, x] = field[base+p+1+x])
    so each [128, K] grid column is a needed pair and the DMA is plain
    contiguous.  Work is split across engines: the Scalar (ACT) engine
    produces min(h_i,h_j) and |p_i - p_j|, DVE the rest; row-sharded over
    8 cores; the host performs the cheap greedy sequential resolution.
"""

import os
import numpy as np

N = 16384
THRESH = 0.5
NCORES = 8
NT = 8                 # 128-row tiles per core
RC = NT * 128          # rows per core
RTOT = NCORES * RC     # padded valid-box capacity (8192)
K = 96                 # device forward band width (realized max offset is 83)
FC = RC + 192          # padded source column span per core
NFIELD = 6             # skw fields: s, e, p, h, a, w
NROWF = 7              # rows fields: s, e, p, h, a, w, -p
NPAIR = NT // 2
K2 = 2 * K
TW = NFIELD * K        # per-tile chunk width in skw layout
ROFF = NT * NROWF      # rows table width, prepended to chunk 0

_cache = {}
last_results = None    # BassKernelResults of the most recent device run


def _build_bass():
    import concourse.bass as bass
    import concourse.mybir as mybir
    from contextlib import ExitStack

    f32 = mybir.dt.float32
    bf16 = mybir.dt.bfloat16
    Alu = mybir.AluOpType
    Act = mybir.ActivationFunctionType
    nc = bass.Bass()
    skw_t = nc.declare_dram_parameter("skw", [128, ROFF + NT * TW], f32, isOutput=False)
    marg_t = nc.declare_dram_parameter("marg", [128, NT * K], bf16, isOutput=True)

    with ExitStack() as ctx:
        skw_sb = ctx.enter_context(
            nc.sbuf_tensor("skw_sb", [128, ROFF + NT * TW], f32)
        )
        out_sb = ctx.enter_context(nc.sbuf_tensor("out_sb", [128, NT * K], bf16))
        pd_sb = ctx.enter_context(nc.sbuf_tensor("pd_sb", [128, NT * K], f32))
        rbuf = ctx.enter_context(nc.sbuf_tensor("rbuf", [128, K], f32))
        # mh is ACT-produced, double-buffered by pair parity
        mh_sb = ctx.enter_context(nc.sbuf_tensor("mh_sb", [128, 2 * K2], f32))
        slab = {
            nm: ctx.enter_context(nc.sbuf_tensor(f"sl_{nm}", [128, K2], f32))
            for nm in ("mxs", "il0", "ia", "ua", "ud", "g", "t1", "t2")
        }
        cin = [ctx.enter_context(nc.semaphore(f"cin{q}")) for q in range(NT)]
        mh_sem = ctx.enter_context(nc.semaphore("mh_sem"))
        pd_sem = ctx.enter_context(nc.semaphore("pd_sem"))
        dve_done = ctx.enter_context(nc.semaphore("dve_done"))
        dma_out = ctx.enter_context(nc.semaphore("dma_out"))
        block = ctx.enter_context(nc.Block())

        def cj(f, t):
            base = ROFF + t * TW + f * K
            return skw_sb[:, base : base + K]

        def ri(f, t):
            return skw_sb[:, t * NROWF + f : t * NROWF + f + 1]

        @block.sync
        def _(sync):
            # chunk t: the contiguous per-tile block of all 6 skewed fields;
            # chunk 0 additionally carries the whole per-row scalar table.
            for q in range(NT):
                lo = 0 if q == 0 else ROFF + q * TW
                hi = ROFF + (q + 1) * TW
                sync.dma_start(
                    out=skw_sb[:, lo:hi], in_=skw_t[:, lo:hi]
                ).then_inc(cin[q], 16)
            for j in range(NPAIR):
                sync.dma_start(
                    out=marg_t[:, j * K2 : (j + 1) * K2],
                    in_=out_sb[:, j * K2 : (j + 1) * K2],
                )._wait_ge(dve_done, j + 1).then_inc(dma_out, 16)
            sync.wait_ge(dma_out, 16 * NPAIR)

        @block.scalar
        def _(scalar):
            # warm the activation table before inputs land
            zero_ap = nc.const_aps.aps[(f32, 0.0)]
            scalar.activation(rbuf[:, :1], zero_ap, Act.Abs, bias=0.0, scale=1.0)

            def emit_mh(j):
                # mh = h_i - relu(h_i - h_j)   == min(h_i, h_j) up to 1 ulp
                par = (j % 2) * K2
                if j >= 2:
                    # parity slab last read by ia of pair j-2
                    scalar.wait_ge(dve_done, j - 1)
                for k, t in enumerate((2 * j, 2 * j + 1)):
                    ins = scalar.activation(
                        rbuf[:], cj(3, t), Act.Relu, bias=ri(3, t), scale=-1.0
                    )
                    ins._wait_ge(cin[t], 16)
                    ins = scalar.activation(
                        mh_sb[:, par + k * K : par + (k + 1) * K],
                        rbuf[:],
                        Act.Abs,
                        bias=ri(3, t),
                        scale=-1.0,
                    )
                ins.then_inc(mh_sem, 1)

            def emit_pd(j, gate=False):
                for k, t in enumerate((2 * j, 2 * j + 1)):
                    ins = scalar.activation(
                        pd_sb[:, t * K : (t + 1) * K],
                        cj(2, t),
                        Act.Abs,
                        bias=ri(6, t),
                        scale=1.0,
                    )
                    if gate:
                        # first ACT data ops: must gate on the input chunks
                        ins._wait_ge(cin[t], 16)
                ins.then_inc(pd_sem, 1)

            # mh(j) immediately precedes pd(j): mh is ready before DVE's ia,
            # pd before DVE's pair tail, and neither gates on later chunks
            for j in range(NPAIR):
                emit_mh(j)
                emit_pd(j)

        @block.vector
        def _(vector):
            for j in range(NPAIR):
                tiles = range(2 * j, 2 * j + 2)
                par = (j % 2) * K2
                for k, t in enumerate(tiles):
                    h = slice(k * K, (k + 1) * K)
                    # mxs = max(s_j, s_i)
                    ins = vector.tensor_scalar_max(slab["mxs"][:, h], cj(0, t), ri(0, t))
                    ins._wait_ge(cin[t], 16)
                    # il0 = min(e_j, e_i) - mxs
                    vector.scalar_tensor_tensor(
                        slab["il0"][:, h], cj(1, t), ri(1, t), slab["mxs"][:, h],
                        Alu.min, Alu.subtract,
                    )
                # ia = relu(il0) * mh            [pair-wide, mh from ACT]
                vector.scalar_tensor_tensor(
                    slab["ia"][:], slab["il0"][:], 0.0, mh_sb[:, par : par + K2],
                    Alu.max, Alu.mult,
                )._wait_ge(mh_sem, j + 1)
                for k, t in enumerate(tiles):
                    h = slice(k * K, (k + 1) * K)
                    # ua = (a_j + a_i) - ia      (union area, > 0)
                    vector.scalar_tensor_tensor(
                        slab["ua"][:, h], cj(4, t), ri(4, t), slab["ia"][:, h],
                        Alu.add, Alu.subtract,
                    )
                    # ud = (w_j + w_i) - il0     (union length, > 0)
                    vector.scalar_tensor_tensor(
                        slab["ud"][:, h], cj(5, t), ri(5, t), slab["il0"][:, h],
                        Alu.add, Alu.subtract,
                    )
                # g = ia - 0.5*ua                [quad-wide]
                vector.scalar_tensor_tensor(
                    slab["g"][:], slab["ua"][:], -0.5, slab["ia"][:], Alu.mult, Alu.add
                )
                vector.tensor_mul(slab["t1"][:], slab["g"][:], slab["ud"][:])
                vector.tensor_mul(
                    slab["t2"][:], pd_sb[:, j * K2 : (j + 1) * K2], slab["ua"][:]
                )._wait_ge(pd_sem, j + 1)
                vector.tensor_sub(
                    out_sb[:, j * K2 : (j + 1) * K2], slab["t1"][:], slab["t2"][:]
                ).then_inc(dve_done, 1)
    return nc


def _get_bass():
    if "nc" not in _cache:
        _cache["nc"] = _build_bass()
    return _cache["nc"]


def _prep_core_inputs(fpad):
    """fpad: [NROWF, >=RC*(NCORES-1)+1+FC] padded start-sorted field table."""
    from numpy.lib.stride_tricks import as_strided

    in_maps = []
    for r in range(NCORES):
        base = r * RC
        buf = np.empty((128, ROFF + NT * TW), np.float32)
        # per-row scalar table, prepended to chunk 0
        buf[:, :ROFF] = (
            fpad[:, base : base + RC]
            .reshape(NROWF, NT, 128)
            .transpose(2, 1, 0)
            .reshape(128, NT * NROWF)
        )
        # skw[p, t, f, c] = fpad[f, base + 1 + t*128 + p + c]   (chunk-major)
        c0 = np.ascontiguousarray(
            fpad[:NFIELD, base + 1 : base + 1 + (NT - 1) * 128 + 127 + K]
        )
        sf, sx = c0.strides
        skw = as_strided(
            c0, shape=(128, NT, NFIELD, K), strides=(sx, 128 * sx, sf, sx)
        )
        buf[:, ROFF:] = skw.reshape(128, NT * TW)
        in_maps.append({"skw": buf})
    return in_maps


def _band_from_margins(margs):
    """margs: list of [128, NT*K] per core (bf16) -> B [RTOT, K] band margins.
    bf16 rounding preserves the margin sign (|S| >> bf16 underflow)."""
    B = np.empty((RTOT, K), np.float32)
    for r in range(NCORES):
        m = np.asarray(margs[r]).astype(np.float32).reshape(128, NT, K)
        B[r * RC : (r + 1) * RC] = m.transpose(1, 0, 2).reshape(RC, K)
    return B


def _host_margin(fi, fj):
    """Exact device-op-order fp32 margin for box rows fi vs fj (dicts of arrays)."""
    f32 = np.float32
    mxs = np.maximum(fi["s"], fj["s"])
    il0 = (np.minimum(fi["e"], fj["e"]) - mxs).astype(f32)
    mh = np.minimum(fi["h"], fj["h"])
    ia = (np.maximum(il0, 0) * mh).astype(f32)
    ua = ((fj["a"] + fi["a"]).astype(f32) - ia).astype(f32)
    pd = np.abs((fj["p"] - fi["p"]).astype(f32))
    ud = ((fj["w"] + fi["w"]).astype(f32) - il0).astype(f32)
    g = ((ua * f32(-0.5)).astype(f32) + ia).astype(f32)
    t1 = (g * ud).astype(f32)
    t2 = (pd * ua).astype(f32)
    return (t1 - t2).astype(f32)


def _residual_pairs(flds, M, kr):
    """Suppression pairs with offset in (K, kr] computed on host (normally none)."""
    if M <= K + 1 or kr <= K:
        return np.empty(0, np.int64), np.empty(0, np.int64)
    u = np.arange(M)[:, None]
    d = np.arange(K + 1, kr + 1)[None, :]
    v = u + d
    ok = v < M
    vc = np.clip(v, 0, M - 1)
    fi = {k: flds[k][u] for k in flds}
    fj = {k: flds[k][vc] for k in flds}
    S = _host_margin(fi, fj)
    su, sd = np.nonzero((S > 0) & ok)
    return su, su + sd + K + 1


def _resolve(M, so, uu, vv):
    """Greedy NMS resolution from suppression pairs (start-order indices)."""
    cu, cv = so[uu], so[vv]
    lo = np.minimum(cu, cv)
    hi = np.maximum(cu, cv)
    o = np.argsort(lo, kind="stable")
    lo, hi = lo[o], hi[o]
    starts = np.searchsorted(lo, np.arange(M + 1))
    keep = np.zeros(M, bool)
    removed = np.zeros(M, bool)
    for rk in range(M):
        if not removed[rk]:
            keep[rk] = True
            removed[hi[starts[rk] : starts[rk + 1]]] = True
    return keep


def _clear_backends():
    try:
        import jax.extend.backend as _jeb

        _jeb.clear_backends()
    except Exception:
        try:
            import jax

            jax.clear_backends()
        except Exception:
            pass


def _ensure_devices():
    """If the caller pinned jax to another platform (leaving fewer than
    NCORES visible devices, which would crash the device run anyway), flip
    to the axon platform. Returns the previous jax_platforms value to
    restore, or None if nothing was changed."""
    try:
        import jax

        if len(jax.devices()) >= NCORES:
            return None
        prev = jax.config.jax_platforms
        jax.config.update("jax_platforms", "axon")
        _clear_backends()
        if len(jax.devices()) >= NCORES:
            return prev
        jax.config.update("jax_platforms", prev)
        _clear_backends()
    except Exception:
        pass
    return None


def kernel(output):
    global last_results
    from concourse.bass_utils import run_bass_kernel_spmd

    output = np.asarray(output, dtype=np.float32)
    conf = output[:, 0]
    order = np.argsort(-conf, kind="stable")
    boxes = output[order]
    M = int((boxes[:, 0] > THRESH).sum())
    assert M <= RTOT, f"valid-box count {M} exceeds kernel capacity {RTOT}"

    V = boxes[:M]
    s = V[:, 1].copy()
    e = V[:, 2].copy()
    p = V[:, 3].copy()
    h = V[:, 4].copy()
    w = (e - s).astype(np.float32)
    a = (w * h).astype(np.float32)
    so = np.argsort(s, kind="stable")            # start-order -> conf rank

    # exact per-input overlap bound: boxes more than maxgap ranks apart are
    # disjoint; the host covers offsets (K, maxgap] (normally none fire)
    ss = s[so]
    maxgap = int((np.searchsorted(ss, ss + np.float32(95.0)) - np.arange(M)).max())

    PAD = RC * (NCORES - 1) + 1 + FC
    fpad = np.zeros((NROWF, max(PAD, RTOT)), np.float32)
    fields = np.stack([s[so], e[so], p[so], h[so], a[so], w[so], -p[so]])
    fpad[:, :M] = fields

    nc = _get_bass()
    in_maps = _prep_core_inputs(fpad)
    trace = bool(int(os.environ.get("NMS_TRACE", "0")))
    prev_platforms = _ensure_devices()
    try:
        res = run_bass_kernel_spmd(nc, in_maps, list(range(NCORES)), trace=trace)
        last_results = res
        margs = [np.asarray(res.results[r]["marg"]) for r in range(NCORES)]
    finally:
        if prev_platforms is not None:
            try:
                import jax

                jax.config.update("jax_platforms", prev_platforms)
                _clear_backends()
            except Exception:
                pass

    B = _band_from_margins(margs)
    uu, dd = np.nonzero(B > 0)
    vv = uu + dd + 1
    ok = (uu < M) & (vv < M)
    uu, vv = uu[ok], vv[ok]
    # residual band (K, maxgap] on host — normally empty for this regime
    flds = {k: fields[i][:M] for i, k in enumerate(("s", "e", "p", "h", "a", "w"))}
    ru, rv = _residual_pairs(flds, M, maxgap)
    uu = np.concatenate([uu, ru])
    vv = np.concatenate([vv, rv])

    keepM = _resolve(M, so, uu, vv)
    keep_full = np.zeros(N, bool)
    keep_full[:M] = keepM
    return boxes[:, 1:] * keep_full[:, None].astype(np.float32)


# revision 60
# speedup vs baseline: 1.0359x; 1.0075x over previous
"""1-D peak-IoU NMS (nn_Detector) on 8 Trainium2 NeuronCores.

Algorithm:
  * Only boxes with conf > 0.5 can be kept or suppress others; after the
    confidence sort they form a prefix of M boxes.
  * piou(i,j) is symmetric and piou > 0.5 requires interval overlap, which
    requires |start_i - start_j| < max_width (95).  Sorting the valid boxes
    by start, every relevant pair lies within a forward rank band.  The
    device computes the suppression margin for the first K=96 forward
    neighbors (the realized max suppression offset for this generator
    regime is 83); the host covers the residual band up to the exact
    per-input overlap bound with the same fp32 formula (normally empty),
    so the kernel is correct for any input.
  * Margin: S > 0  <=>  piou > 0.5 via the exact-sign division-free form
    S = (ia - ua/2)*ud - pd*ua   (ua, ud > 0).
  * The inputs are pre-skewed on the host (skw[p, x] = field[base+p+1+x])
    so each [128, K] grid column is a needed pair and the DMA is plain
    contiguous.  Work is split across engines: the Scalar (ACT) engine
    produces min(h_i,h_j) and |p_i - p_j|, DVE the rest; row-sharded over
    8 cores; the host performs the cheap greedy sequential resolution.
"""

import os
import numpy as np

N = 16384
THRESH = 0.5
NCORES = 8
NT = 8                 # 128-row tiles per core
RC = NT * 128          # rows per core
RTOT = NCORES * RC     # padded valid-box capacity (8192)
K = 88                 # device forward band width (realized max offset is 83)
FC = RC + 192          # padded source column span per core
NFIELD = 6             # skw fields: s, e, p, h, a, w
NROWF = 7              # rows fields: s, e, p, h, a, w, -p
NPAIR = NT // 2
K2 = 2 * K
TW = NFIELD * K        # per-tile chunk width in skw layout
ROFF = NT * NROWF      # rows table width, prepended to chunk 0

_cache = {}
last_results = None    # BassKernelResults of the most recent device run


def _build_bass():
    import concourse.bass as bass
    import concourse.mybir as mybir
    from contextlib import ExitStack

    f32 = mybir.dt.float32
    bf16 = mybir.dt.bfloat16
    Alu = mybir.AluOpType
    Act = mybir.ActivationFunctionType
    nc = bass.Bass()
    skw_t = nc.declare_dram_parameter("skw", [128, ROFF + NT * TW], f32, isOutput=False)
    marg_t = nc.declare_dram_parameter("marg", [128, NT * K], bf16, isOutput=True)

    with ExitStack() as ctx:
        skw_sb = ctx.enter_context(
            nc.sbuf_tensor("skw_sb", [128, ROFF + NT * TW], f32)
        )
        out_sb = ctx.enter_context(nc.sbuf_tensor("out_sb", [128, NT * K], bf16))
        pd_sb = ctx.enter_context(nc.sbuf_tensor("pd_sb", [128, NT * K], f32))
        rbuf = ctx.enter_context(nc.sbuf_tensor("rbuf", [128, K], f32))
        # mh is ACT-produced, double-buffered by pair parity
        mh_sb = ctx.enter_context(nc.sbuf_tensor("mh_sb", [128, 2 * K2], f32))
        slab = {
            nm: ctx.enter_context(nc.sbuf_tensor(f"sl_{nm}", [128, K2], f32))
            for nm in ("mxs", "il0", "ia", "ua", "ud", "g", "t1", "t2")
        }
        cin = [ctx.enter_context(nc.semaphore(f"cin{q}")) for q in range(NT)]
        mh_sem = ctx.enter_context(nc.semaphore("mh_sem"))
        pd_sem = ctx.enter_context(nc.semaphore("pd_sem"))
        dve_done = ctx.enter_context(nc.semaphore("dve_done"))
        dma_out = ctx.enter_context(nc.semaphore("dma_out"))
        block = ctx.enter_context(nc.Block())

        def cj(f, t):
            base = ROFF + t * TW + f * K
            return skw_sb[:, base : base + K]

        def ri(f, t):
            return skw_sb[:, t * NROWF + f : t * NROWF + f + 1]

        @block.sync
        def _(sync):
            # chunk t: the contiguous per-tile block of all 6 skewed fields;
            # chunk 0 additionally carries the whole per-row scalar table.
            for q in range(NT):
                lo = 0 if q == 0 else ROFF + q * TW
                hi = ROFF + (q + 1) * TW
                sync.dma_start(
                    out=skw_sb[:, lo:hi], in_=skw_t[:, lo:hi]
                ).then_inc(cin[q], 16)
            for j in range(NPAIR):
                sync.dma_start(
                    out=marg_t[:, j * K2 : (j + 1) * K2],
                    in_=out_sb[:, j * K2 : (j + 1) * K2],
                )._wait_ge(dve_done, j + 1).then_inc(dma_out, 16)
            sync.wait_ge(dma_out, 16 * NPAIR)

        @block.scalar
        def _(scalar):
            # warm the activation table before inputs land
            zero_ap = nc.const_aps.aps[(f32, 0.0)]
            scalar.activation(rbuf[:, :1], zero_ap, Act.Abs, bias=0.0, scale=1.0)

            def emit_mh(j):
                # mh = h_i - relu(h_i - h_j)   == min(h_i, h_j) up to 1 ulp
                par = (j % 2) * K2
                if j >= 2:
                    # parity slab last read by ia of pair j-2
                    scalar.wait_ge(dve_done, j - 1)
                for k, t in enumerate((2 * j, 2 * j + 1)):
                    ins = scalar.activation(
                        rbuf[:], cj(3, t), Act.Relu, bias=ri(3, t), scale=-1.0
                    )
                    ins._wait_ge(cin[t], 16)
                    ins = scalar.activation(
                        mh_sb[:, par + k * K : par + (k + 1) * K],
                        rbuf[:],
                        Act.Abs,
                        bias=ri(3, t),
                        scale=-1.0,
                    )
                ins.then_inc(mh_sem, 1)

            def emit_pd(j, gate=False):
                for k, t in enumerate((2 * j, 2 * j + 1)):
                    ins = scalar.activation(
                        pd_sb[:, t * K : (t + 1) * K],
                        cj(2, t),
                        Act.Abs,
                        bias=ri(6, t),
                        scale=1.0,
                    )
                    if gate:
                        # first ACT data ops: must gate on the input chunks
                        ins._wait_ge(cin[t], 16)
                ins.then_inc(pd_sem, 1)

            # mh(j) immediately precedes pd(j): mh is ready before DVE's ia,
            # pd before DVE's pair tail, and neither gates on later chunks
            for j in range(NPAIR):
                emit_mh(j)
                emit_pd(j)

        @block.vector
        def _(vector):
            for j in range(NPAIR):
                tiles = range(2 * j, 2 * j + 2)
                par = (j % 2) * K2
                for k, t in enumerate(tiles):
                    h = slice(k * K, (k + 1) * K)
                    # mxs = max(s_j, s_i)
                    ins = vector.tensor_scalar_max(slab["mxs"][:, h], cj(0, t), ri(0, t))
                    ins._wait_ge(cin[t], 16)
                    # il0 = min(e_j, e_i) - mxs
                    vector.scalar_tensor_tensor(
                        slab["il0"][:, h], cj(1, t), ri(1, t), slab["mxs"][:, h],
                        Alu.min, Alu.subtract,
                    )
                # ia = relu(il0) * mh            [pair-wide, mh from ACT]
                vector.scalar_tensor_tensor(
                    slab["ia"][:], slab["il0"][:], 0.0, mh_sb[:, par : par + K2],
                    Alu.max, Alu.mult,
                )._wait_ge(mh_sem, j + 1)
                for k, t in enumerate(tiles):
                    h = slice(k * K, (k + 1) * K)
                    # ua = (a_j + a_i) - ia      (union area, > 0)
                    vector.scalar_tensor_tensor(
                        slab["ua"][:, h], cj(4, t), ri(4, t), slab["ia"][:, h],
                        Alu.add, Alu.subtract,
                    )
                    # ud = (w_j + w_i) - il0     (union length, > 0)
                    vector.scalar_tensor_tensor(
                        slab["ud"][:, h], cj(5, t), ri(5, t), slab["il0"][:, h],
                        Alu.add, Alu.subtract,
                    )
                # g = ia - 0.5*ua                [quad-wide]
                vector.scalar_tensor_tensor(
                    slab["g"][:], slab["ua"][:], -0.5, slab["ia"][:], Alu.mult, Alu.add
                )
                vector.tensor_mul(slab["t1"][:], slab["g"][:], slab["ud"][:])
                vector.tensor_mul(
                    slab["t2"][:], pd_sb[:, j * K2 : (j + 1) * K2], slab["ua"][:]
                )._wait_ge(pd_sem, j + 1)
                vector.tensor_sub(
                    out_sb[:, j * K2 : (j + 1) * K2], slab["t1"][:], slab["t2"][:]
                ).then_inc(dve_done, 1)
    return nc


def _get_bass():
    if "nc" not in _cache:
        _cache["nc"] = _build_bass()
    return _cache["nc"]


def _prep_core_inputs(fpad):
    """fpad: [NROWF, >=RC*(NCORES-1)+1+FC] padded start-sorted field table."""
    from numpy.lib.stride_tricks import as_strided

    in_maps = []
    for r in range(NCORES):
        base = r * RC
        buf = np.empty((128, ROFF + NT * TW), np.float32)
        # per-row scalar table, prepended to chunk 0
        buf[:, :ROFF] = (
            fpad[:, base : base + RC]
            .reshape(NROWF, NT, 128)
            .transpose(2, 1, 0)
            .reshape(128, NT * NROWF)
        )
        # skw[p, t, f, c] = fpad[f, base + 1 + t*128 + p + c]   (chunk-major)
        c0 = np.ascontiguousarray(
            fpad[:NFIELD, base + 1 : base + 1 + (NT - 1) * 128 + 127 + K]
        )
        sf, sx = c0.strides
        skw = as_strided(
            c0, shape=(128, NT, NFIELD, K), strides=(sx, 128 * sx, sf, sx)
        )
        buf[:, ROFF:] = skw.reshape(128, NT * TW)
        in_maps.append({"skw": buf})
    return in_maps


def _band_from_margins(margs):
    """margs: list of [128, NT*K] per core (bf16) -> B [RTOT, K] band margins.
    bf16 rounding preserves the margin sign (|S| >> bf16 underflow)."""
    B = np.empty((RTOT, K), np.float32)
    for r in range(NCORES):
        m = np.asarray(margs[r]).astype(np.float32).reshape(128, NT, K)
        B[r * RC : (r + 1) * RC] = m.transpose(1, 0, 2).reshape(RC, K)
    return B


def _host_margin(fi, fj):
    """Exact device-op-order fp32 margin for box rows fi vs fj (dicts of arrays)."""
    f32 = np.float32
    mxs = np.maximum(fi["s"], fj["s"])
    il0 = (np.minimum(fi["e"], fj["e"]) - mxs).astype(f32)
    mh = np.minimum(fi["h"], fj["h"])
    ia = (np.maximum(il0, 0) * mh).astype(f32)
    ua = ((fj["a"] + fi["a"]).astype(f32) - ia).astype(f32)
    pd = np.abs((fj["p"] - fi["p"]).astype(f32))
    ud = ((fj["w"] + fi["w"]).astype(f32) - il0).astype(f32)
    g = ((ua * f32(-0.5)).astype(f32) + ia).astype(f32)
    t1 = (g * ud).astype(f32)
    t2 = (pd * ua).astype(f32)
    return (t1 - t2).astype(f32)


def _residual_pairs(flds, M, kr):
    """Suppression pairs with offset in (K, kr] computed on host (normally none)."""
    if M <= K + 1 or kr <= K:
        return np.empty(0, np.int64), np.empty(0, np.int64)
    u = np.arange(M)[:, None]
    d = np.arange(K + 1, kr + 1)[None, :]
    v = u + d
    ok = v < M
    vc = np.clip(v, 0, M - 1)
    fi = {k: flds[k][u] for k in flds}
    fj = {k: flds[k][vc] for k in flds}
    S = _host_margin(fi, fj)
    su, sd = np.nonzero((S > 0) & ok)
    return su, su + sd + K + 1


def _resolve(M, so, uu, vv):
    """Greedy NMS resolution from suppression pairs (start-order indices)."""
    cu, cv = so[uu], so[vv]
    lo = np.minimum(cu, cv)
    hi = np.maximum(cu, cv)
    o = np.argsort(lo, kind="stable")
    lo, hi = lo[o], hi[o]
    starts = np.searchsorted(lo, np.arange(M + 1))
    keep = np.zeros(M, bool)
    removed = np.zeros(M, bool)
    for rk in range(M):
        if not removed[rk]:
            keep[rk] = True
            removed[hi[starts[rk] : starts[rk + 1]]] = True
    return keep


def _clear_backends():
    try:
        import jax.extend.backend as _jeb

        _jeb.clear_backends()
    except Exception:
        try:
            import jax

            jax.clear_backends()
        except Exception:
            pass


def _ensure_devices():
    """If the caller pinned jax to another platform (leaving fewer than
    NCORES visible devices, which would crash the device run anyway), flip
    to the axon platform. Returns the previous jax_platforms value to
    restore, or None if nothing was changed."""
    try:
        import jax

        if len(jax.devices()) >= NCORES:
            return None
        prev = jax.config.jax_platforms
        jax.config.update("jax_platforms", "axon")
        _clear_backends()
        if len(jax.devices()) >= NCORES:
            return prev
        jax.config.update("jax_platforms", prev)
        _clear_backends()
    except Exception:
        pass
    return None


def kernel(output):
    global last_results
    from concourse.bass_utils import run_bass_kernel_spmd

    output = np.asarray(output, dtype=np.float32)
    conf = output[:, 0]
    order = np.argsort(-conf, kind="stable")
    boxes = output[order]
    M = int((boxes[:, 0] > THRESH).sum())
    assert M <= RTOT, f"valid-box count {M} exceeds kernel capacity {RTOT}"

    V = boxes[:M]
    s = V[:, 1].copy()
    e = V[:, 2].copy()
    p = V[:, 3].copy()
    h = V[:, 4].copy()
    w = (e - s).astype(np.float32)
    a = (w * h).astype(np.float32)
    so = np.argsort(s, kind="stable")            # start-order -> conf rank

    # exact per-input overlap bound: boxes more than maxgap ranks apart are
    # disjoint; the host covers offsets (K, maxgap] (normally none fire)
    ss = s[so]
    maxgap = int((np.searchsorted(ss, ss + np.float32(95.0)) - np.arange(M)).max())

    PAD = RC * (NCORES - 1) + 1 + FC
    fpad = np.zeros((NROWF, max(PAD, RTOT)), np.float32)
    fields = np.stack([s[so], e[so], p[so], h[so], a[so], w[so], -p[so]])
    fpad[:, :M] = fields

    nc = _get_bass()
    in_maps = _prep_core_inputs(fpad)
    trace = bool(int(os.environ.get("NMS_TRACE", "0")))
    prev_platforms = _ensure_devices()
    try:
        res = run_bass_kernel_spmd(nc, in_maps, list(range(NCORES)), trace=trace)
        last_results = res
        margs = [np.asarray(res.results[r]["marg"]) for r in range(NCORES)]
    finally:
        if prev_platforms is not None:
            try:
                import jax

                jax.config.update("jax_platforms", prev_platforms)
                _clear_backends()
            except Exception:
                pass

    B = _band_from_margins(margs)
    uu, dd = np.nonzero(B > 0)
    vv = uu + dd + 1
    ok = (uu < M) & (vv < M)
    uu, vv = uu[ok], vv[ok]
    # residual band (K, maxgap] on host — normally empty for this regime
    flds = {k: fields[i][:M] for i, k in enumerate(("s", "e", "p", "h", "a", "w"))}
    ru, rv = _residual_pairs(flds, M, maxgap)
    uu = np.concatenate([uu, ru])
    vv = np.concatenate([vv, rv])

    keepM = _resolve(M, so, uu, vv)
    keep_full = np.zeros(N, bool)
    keep_full[:M] = keepM
    return boxes[:, 1:] * keep_full[:, None].astype(np.float32)
